# revision 5
# baseline (speedup 1.0000x reference)
"""Trainium2 Bass kernel for nn_Clustering_80900003987951 (vq_codebook).

Math (reference):
  x: [B=128, S=128, F=64, 1], centroids: [1, K=64, S=128, F=64]
  d2[b,k,s] = sum_f (x[b,s,f] - c[k,s,f])^2
  dist[b,k] = sum_s sqrt(d2[b,k,s])
  q = (1 + dist^2/2)^-3, normalized over k                  -> [B, K]

Strategy: shard the SEQUENCE dim across the 8 cores (S_loc=16), keep the
full batch on every core. Per-core input drops to ~400KB (vs 1.36MB for
batch sharding, where every core must load all centroids), matmuls use
all 128 output partitions, and the device returns the per-core partial
  qp[b,k] = sum_{s in shard} sqrt(d2[b,k,s])     [128, 64] f32
The host sums the 8 partials and applies the tiny q tail (25K flops,
~0.002% of the work) exactly in float64.

Device pipeline per core:
  xt [66, S_loc*B]: rows 0-63 = x^T (F on partitions), 64 = 1, 65 = |x|^2
  ct [66, S_loc*K]: rows 0-63 = -2*c^T, 64 = |c|^2, 65 = 1
  per s: d2 tile = xt_s^T @ ct_s  -> PSUM [128, 64]
  16 matmuls split over 4 PSUM banks; ACT sqrt per bank -> fp16 sbuf;
  contiguous fp16 add-tree on DVE for the s-sum (a strided tensor_reduce
  measured 1.8ns/elem vs ~0.5 for contiguous fp16 adds); DMA out.
DMA notes: queue throughput is descriptor-bound (~22ns/descriptor, one
descriptor per partition row), so xt goes as ONE 66-descriptor transfer
with 4KB rows on the sync queue and ct (66 x 2KB) on the scalar queue.
"""

import numpy as np

B, K, S, F = 128, 64, 128, 64
NCORES = 8
SLOC = S // NCORES          # 16 sequence positions per core
NBANK = 4                   # psum banks used
TB = SLOC // NBANK          # 4 s-positions per bank
CP = F + 2                  # 66 contraction rows (data + aug)

# fp16 operands: halves DMA bytes and avoids the 2-pass fp32 PE matmul;
# fp32 PSUM accumulation keeps the error ~1e-4.
XT_DT = "float16"
CT_DT = "float16"
DI_DT = "float16"           # sqrt results + partial sums; 2x DVE throughput

_CACHE = {}


def _build_nc():
    import concourse.bacc as bacc
    import concourse.tile as tile
    from concourse import mybir

    f32 = mybir.dt.float32
    fxt = getattr(mybir.dt, XT_DT)
    fct = getattr(mybir.dt, CT_DT)
    fdi = getattr(mybir.dt, DI_DT)
    nc = bacc.Bacc("TRN2", target_bir_lowering=False, debug=False)

    xt_d = nc.dram_tensor("xt", [CP, SLOC * B], fxt, kind="ExternalInput")
    ct_d = nc.dram_tensor("ct", [CP, SLOC * K], fct, kind="ExternalInput")
    qp_d = nc.dram_tensor("qp", [B, K], f32, kind="ExternalOutput")

    with tile.TileContext(nc) as tc:
        with (
            tc.tile_pool(name="ins", bufs=1) as in_pool,
            tc.tile_pool(name="psum", bufs=1, space="PSUM") as psum_pool,
            tc.tile_pool(name="work", bufs=1) as work_pool,
        ):
            xt_t = in_pool.tile([CP, SLOC * B], fxt, name="xtt")
            ct_t = in_pool.tile([CP, SLOC * K], fct, name="ctt")
            # 2 parallel DGE queues (only SP/Activation can issue HWDGE
            # DMAs; gpsimd SWDGE crashed the exec unit).
            nc.sync.dma_start(out=xt_t[:], in_=xt_d.ap())
            nc.scalar.dma_start(out=ct_t[:], in_=ct_d.ap())

            pss = [
                psum_pool.tile([128, TB * K], f32, name=f"ps{b}")
                for b in range(NBANK)
            ]
            dis = [
                work_pool.tile([128, TB, K], fdi, name=f"di{b}")
                for b in range(NBANK)
            ]

            for s in range(SLOC):
                b, u = divmod(s, TB)
                nc.tensor.matmul(
                    pss[b][:, u * K:(u + 1) * K],
                    lhsT=xt_t[:, s * B:(s + 1) * B],
                    rhs=ct_t[:, s * K:(s + 1) * K],
                    start=True,
                    stop=True,
                )

            # per-bank: sqrt (ACT) then a contiguous fp16 add-tree (DVE)
            pbs = []
            for b in range(NBANK):
                nc.scalar.activation(
                    dis[b][:], pss[b][:], mybir.ActivationFunctionType.Sqrt
                )
                tb = work_pool.tile([128, 2, K], fdi, name=f"tb{b}")
                nc.vector.tensor_tensor(
                    tb[:], dis[b][:, 0:2, :], dis[b][:, 2:4, :],
                    op=mybir.AluOpType.add,
                )
                pb = work_pool.tile([128, K], fdi, name=f"pb{b}")
                nc.vector.tensor_tensor(
                    pb[:], tb[:, 0, :], tb[:, 1, :], op=mybir.AluOpType.add
                )
                pbs.append(pb)

            p01 = work_pool.tile([128, K], fdi, name="p01")
            nc.vector.tensor_tensor(
                p01[:], pbs[0][:], pbs[1][:], op=mybir.AluOpType.add
            )
            p23 = work_pool.tile([128, K], fdi, name="p23")
            nc.vector.tensor_tensor(
                p23[:], pbs[2][:], pbs[3][:], op=mybir.AluOpType.add
            )
            qt = work_pool.tile([B, K], f32, name="qt")
            nc.vector.tensor_tensor(
                qt[:], p01[:], p23[:], op=mybir.AluOpType.add
            )
            nc.sync.dma_start(out=qp_d.ap(), in_=qt[:])

    nc.compile()
    return nc


def _prep_inputs(x, centroids):
    """Host-side shard + transpose + augmentation. Returns in_maps list."""
    from concourse import mybir

    xt_np = mybir.dt.np(getattr(mybir.dt, XT_DT))
    ct_np = mybir.dt.np(getattr(mybir.dt, CT_DT))
    x = np.ascontiguousarray(np.asarray(x, dtype=np.float32)).reshape(B, S, F)
    c = np.ascontiguousarray(np.asarray(centroids, dtype=np.float32)).reshape(K, S, F)

    in_maps = []
    for i in range(NCORES):
        sl = slice(i * SLOC, (i + 1) * SLOC)
        xs = x[:, sl, :]                              # [B, SLOC, F]
        xt = np.empty((CP, SLOC * B), dtype=xt_np)
        xt[:F] = xs.transpose(2, 1, 0).reshape(F, SLOC * B)
        xt[F] = 1.0
        xt[F + 1] = ((xs * xs).sum(-1, dtype=np.float32).T).reshape(SLOC * B)
        cs = c[:, sl, :]                              # [K, SLOC, F]
        ct = np.empty((CP, SLOC * K), dtype=ct_np)
        ct[:F] = (-2.0 * cs).transpose(2, 1, 0).reshape(F, SLOC * K)
        ct[F] = ((cs * cs).sum(-1, dtype=np.float32).T).reshape(SLOC * K)
        ct[F + 1] = 1.0
        in_maps.append({"xt": xt, "ct": ct})
    return in_maps


def kernel(x, centroids):
    from concourse.bass_utils import run_bass_kernel_spmd

    if "nc" not in _CACHE:
        _CACHE["nc"] = _build_nc()
    nc = _CACHE["nc"]

    in_maps = _prep_inputs(x, centroids)
    res = run_bass_kernel_spmd(nc, in_maps, core_ids=list(range(NCORES)))
    dist = np.zeros((B, K), dtype=np.float64)
    for i in range(NCORES):
        dist += res.results[i]["qp"].astype(np.float64)
    # q tail (exact, host): q = (1 + d^2/2)^-3 normalized over k
    q = 1.0 / (1.0 + dist * dist / 2.0)
    q = q * q * q
    q = q / q.sum(axis=1, keepdims=True)
    return q.astype(np.float32)


# revision 6
# speedup vs baseline: 1.0663x; 1.0663x over previous
"""Trainium2 Bass kernel for nn_Clustering_80900003987951 (vq_codebook).

Math (reference):
  x: [B=128, S=128, F=64, 1], centroids: [1, K=64, S=128, F=64]
  d2[b,k,s] = sum_f (x[b,s,f] - c[k,s,f])^2
  dist[b,k] = sum_s sqrt(d2[b,k,s])
  q = (1 + dist^2/2)^-3, normalized over k                  -> [B, K]

Strategy: shard the SEQUENCE dim across the 8 cores (S_loc=16), keep the
full batch on every core. Per-core input drops to ~200KB (vs 1.36MB for
batch sharding, where every core must load all centroids), matmuls use
all 128 output partitions, and the device returns the per-core partial
  qp[b,k] = sum_{s in shard} sqrt(d2[b,k,s])     [128, 64] f32
The host sums the 8 partials and applies the tiny q tail (25K flops,
~0.002% of the work) exactly in float64.

Device pipeline per core, using d2 = (-2<x,c>) + |x|^2 + |c|^2:
  xt8 [64, S_loc*B] fp8:  x^T   (cross term operand, col = s*B+b)
  ct8 [64, S_loc*K] fp8:  -2*c^T                    (col = s*K+k)
  aug [2, S_loc*(B+K)] fp16: rows (1,|x|^2) for x cols / (|c|^2,1) for c
  per s: TWO accumulating matmuls into one PSUM [128,64] region:
    fp8  [64-row]: cross term; fp16 [2-row]: exact |x|^2 + |c|^2.
  (fp8 only touches the cross term -> measured-scale error ~2e-3, while
   fp8 |x|^2 rows would have cost ~1e-2.)
  16 s split over 4 PSUM banks; ACT sqrt per bank -> fp16 sbuf;
  contiguous fp16 add-tree + running accumulator on DVE for the s-sum
  (a strided tensor_reduce measured 1.8ns/elem vs ~0.5 contiguous); DMA
  the [128,64] partial out.
DMA notes: per-queue throughput is bytes-bound (~85GB/s measured), so the
fp8 halving is real; transfers are split so bank-0 operands land first
and matmuls overlap the remaining stream.
"""

import numpy as np

B, K, S, F = 128, 64, 128, 64
NCORES = 8
SLOC = S // NCORES          # 16 sequence positions per core
NBANK = 4                   # psum banks used
TB = SLOC // NBANK          # 4 s-positions per bank

X8_DT = "float8e4"          # cross-term operands
C8_DT = "float8e4"
AUG_DT = "float16"          # |x|^2 / |c|^2 augmentation rows
DI_DT = "float16"           # sqrt results + partial sums; 2x DVE throughput
USE_GPSIMD_DMA = True       # third DGE queue for the second xt half

_CACHE = {}


def _build_nc():
    import concourse.bacc as bacc
    import concourse.tile as tile
    from concourse import mybir

    f32 = mybir.dt.float32
    fx8 = getattr(mybir.dt, X8_DT)
    fc8 = getattr(mybir.dt, C8_DT)
    fau = getattr(mybir.dt, AUG_DT)
    fdi = getattr(mybir.dt, DI_DT)
    nc = bacc.Bacc("TRN2", target_bir_lowering=False, debug=False)

    HB = SLOC * B // 2      # 1024 cols per xt half
    xt0_d = nc.dram_tensor("xt0", [F, HB], fx8, kind="ExternalInput")
    xt1_d = nc.dram_tensor("xt1", [F, HB], fx8, kind="ExternalInput")
    ct_d = nc.dram_tensor("ct", [F, SLOC * K], fc8, kind="ExternalInput")
    # aug cols: [0, SLOC*B) = x side (rows: 1, |x|^2), col s*B+b
    #           [SLOC*B, SLOC*(B+K)) = c side (rows: |c|^2, 1), col s*K+k
    aug_d = nc.dram_tensor(
        "aug", [2, SLOC * (B + K)], fau, kind="ExternalInput"
    )
    qp_d = nc.dram_tensor("qp", [B, K], f32, kind="ExternalOutput")

    with tile.TileContext(nc) as tc:
        with (
            tc.tile_pool(name="ins", bufs=1) as in_pool,
            tc.tile_pool(name="psum", bufs=1, space="PSUM") as psum_pool,
            tc.tile_pool(name="work", bufs=1) as work_pool,
        ):
            xt0_t = in_pool.tile([F, HB], fx8, name="xt0t")
            xt1_t = in_pool.tile([F, HB], fx8, name="xt1t")
            ct_t = in_pool.tile([F, SLOC * K], fc8, name="ctt")
            aug_t = in_pool.tile([2, SLOC * (B + K)], fau, name="augt")
            nc.sync.dma_start(out=xt0_t[:], in_=xt0_d.ap())
            nc.scalar.dma_start(out=ct_t[:], in_=ct_d.ap())
            if USE_GPSIMD_DMA:
                nc.gpsimd.dma_start(out=xt1_t[:], in_=xt1_d.ap())
            else:
                nc.sync.dma_start(out=xt1_t[:], in_=xt1_d.ap())
            nc.scalar.dma_start(out=aug_t[:], in_=aug_d.ap())

            pss = [
                psum_pool.tile([128, TB * K], f32, name=f"ps{b}")
                for b in range(NBANK)
            ]
            dis = [
                work_pool.tile([128, TB, K], fdi, name=f"di{b}")
                for b in range(NBANK)
            ]

            xts = (xt0_t, xt1_t)
            for s in range(SLOC):
                b, u = divmod(s, TB)
                xt_t = xts[s * B // HB]
                xo = (s * B) % HB
                out = pss[b][:, u * K:(u + 1) * K]
                nc.tensor.matmul(
                    out,
                    lhsT=xt_t[:, xo:xo + B],
                    rhs=ct_t[:, s * K:(s + 1) * K],
                    start=True,
                    stop=False,
                )
                nc.tensor.matmul(
                    out,
                    lhsT=aug_t[:, s * B:(s + 1) * B],
                    rhs=aug_t[:, SLOC * B + s * K:SLOC * B + (s + 1) * K],
                    start=False,
                    stop=True,
                )

            # per-bank: sqrt (ACT), contiguous fp16 add-tree + running
            # accumulator (DVE)
            acc = None
            for b in range(NBANK):
                nc.scalar.activation(
                    dis[b][:], pss[b][:], mybir.ActivationFunctionType.Sqrt
                )
                tb = work_pool.tile([128, 2, K], fdi, name=f"tb{b}")
                nc.vector.tensor_tensor(
                    tb[:], dis[b][:, 0:2, :], dis[b][:, 2:4, :],
                    op=mybir.AluOpType.add,
                )
                pb = work_pool.tile(
                    [128, K], fdi if b < NBANK - 1 else fdi, name=f"pb{b}"
                )
                nc.vector.tensor_tensor(
                    pb[:], tb[:, 0, :], tb[:, 1, :], op=mybir.AluOpType.add
                )
                if acc is None:
                    acc = pb
                elif b < NBANK - 1:
                    nacc = work_pool.tile([128, K], fdi, name=f"acc{b}")
                    nc.vector.tensor_tensor(
                        nacc[:], acc[:], pb[:], op=mybir.AluOpType.add
                    )
                    acc = nacc
                else:
                    qt = work_pool.tile([B, K], f32, name="qt")
                    nc.vector.tensor_tensor(
                        qt[:], acc[:], pb[:], op=mybir.AluOpType.add
                    )
                    acc = qt
            nc.sync.dma_start(out=qp_d.ap(), in_=acc[:])

    nc.compile()
    return nc


def _prep_inputs(x, centroids):
    """Host-side shard + transpose + augmentation. Returns in_maps list."""
    from concourse import mybir

    x8_np = mybir.dt.np(getattr(mybir.dt, X8_DT))
    c8_np = mybir.dt.np(getattr(mybir.dt, C8_DT))
    au_np = mybir.dt.np(getattr(mybir.dt, AUG_DT))
    x = np.ascontiguousarray(np.asarray(x, dtype=np.float32)).reshape(B, S, F)
    c = np.ascontiguousarray(np.asarray(centroids, dtype=np.float32)).reshape(K, S, F)

    HB = SLOC * B // 2
    in_maps = []
    for i in range(NCORES):
        sl = slice(i * SLOC, (i + 1) * SLOC)
        xs = x[:, sl, :]                              # [B, SLOC, F]
        xt = xs.transpose(2, 1, 0).reshape(F, SLOC * B).astype(x8_np)
        cs = c[:, sl, :]                              # [K, SLOC, F]
        ct = (-2.0 * cs).transpose(2, 1, 0).reshape(F, SLOC * K).astype(c8_np)
        aug = np.empty((2, SLOC * (B + K)), dtype=au_np)
        aug[0, :SLOC * B] = 1.0
        aug[1, :SLOC * B] = (
            (xs * xs).sum(-1, dtype=np.float32).T
        ).reshape(SLOC * B)
        aug[0, SLOC * B:] = (
            (cs * cs).sum(-1, dtype=np.float32).T
        ).reshape(SLOC * K)
        aug[1, SLOC * B:] = 1.0
        in_maps.append({
            "xt0": np.ascontiguousarray(xt[:, :HB]),
            "xt1": np.ascontiguousarray(xt[:, HB:]),
            "ct": ct,
            "aug": aug,
        })
    return in_maps


def kernel(x, centroids):
    from concourse.bass_utils import run_bass_kernel_spmd

    if "nc" not in _CACHE:
        _CACHE["nc"] = _build_nc()
    nc = _CACHE["nc"]

    in_maps = _prep_inputs(x, centroids)
    res = run_bass_kernel_spmd(nc, in_maps, core_ids=list(range(NCORES)))
    dist = np.zeros((B, K), dtype=np.float64)
    for i in range(NCORES):
        dist += res.results[i]["qp"].astype(np.float64)
    # q tail (exact, host): q = (1 + d^2/2)^-3 normalized over k
    q = 1.0 / (1.0 + dist * dist / 2.0)
    q = q * q * q
    q = q / q.sum(axis=1, keepdims=True)
    return q.astype(np.float32)


# revision 7
# speedup vs baseline: 1.1922x; 1.1181x over previous
"""Trainium2 Bass kernel for nn_Clustering_80900003987951 (vq_codebook).

Math (reference):
  x: [B=128, S=128, F=64, 1], centroids: [1, K=64, S=128, F=64]
  d2[b,k,s] = sum_f (x[b,s,f] - c[k,s,f])^2
  dist[b,k] = sum_s sqrt(d2[b,k,s])
  q = (1 + dist^2/2)^-3, normalized over k                  -> [B, K]

Strategy: shard the SEQUENCE dim across the 8 cores (S_loc=16), keep the
full batch on every core. Per-core input drops to ~400KB (vs 1.36MB for
batch sharding, where every core must load all centroids), matmuls use
all 128 output partitions, and the device returns the per-core partial
  qp[b,k] = sum_{s in shard} sqrt(d2[b,k,s])     [128, 64] f32
The host sums the 8 partials and applies the tiny q tail (25K flops,
~0.002% of the work) exactly in float64.

Device pipeline per core:
  xt [66, S_loc*B]: rows 0-63 = x^T (F on partitions), 64 = 1, 65 = |x|^2
  ct [66, S_loc*K]: rows 0-63 = -2*c^T, 64 = |c|^2, 65 = 1
  per s: d2 tile = xt_s^T @ ct_s -> PSUM [128,64]  (fp16; one matmul per
  s — mixing fp8 cross-term + fp16 aug matmuls measured 325ns/s because
  alternating weight dtypes breaks ldweights/matmul pipelining, vs 60ns
  here)
  16 s split over 4 PSUM banks; ACT sqrt per bank -> fp16 sbuf;
  contiguous fp16 add-tree + running accumulator on DVE for the s-sum
  (a strided tensor_reduce measured 1.8ns/elem vs ~0.5 contiguous).
DMA notes: per-queue throughput is ~85-135GB/s regardless of descriptor
size, and only SP/Activation issue usable HWDGE queues (gpsimd SWDGE
moved 64KB in ~4us). Transfers are split and interleaved so bank-0
operands land first and later banks stream in behind the matmuls:
  sync:   xt0 (s0-7, 135KB)  then ct1 (s8-15, 67KB)
  scalar: ct0 (s0-7,  67KB)  then xt1 (s8-15, 135KB)
"""

import numpy as np

B, K, S, F = 128, 64, 128, 64
NCORES = 8
SLOC = S // NCORES          # 16 sequence positions per core
NBANK = 4                   # psum banks used
TB = SLOC // NBANK          # 4 s-positions per bank
CP = F + 2                  # 66 contraction rows (data + aug)
HS = SLOC // 2              # 8: s-positions per transfer piece

XT_DT = "float16"
CT_DT = "float16"
DI_DT = "float16"           # sqrt results + partial sums; 2x DVE throughput

_CACHE = {}


def _build_nc():
    import concourse.bacc as bacc
    import concourse.tile as tile
    from concourse import mybir

    f32 = mybir.dt.float32
    fxt = getattr(mybir.dt, XT_DT)
    fct = getattr(mybir.dt, CT_DT)
    fdi = getattr(mybir.dt, DI_DT)
    nc = bacc.Bacc("TRN2", target_bir_lowering=False, debug=False)

    xt0_d = nc.dram_tensor("xt0", [CP, HS * B], fxt, kind="ExternalInput")
    xt1_d = nc.dram_tensor("xt1", [CP, HS * B], fxt, kind="ExternalInput")
    ct0_d = nc.dram_tensor("ct0", [CP, HS * K], fct, kind="ExternalInput")
    ct1_d = nc.dram_tensor("ct1", [CP, HS * K], fct, kind="ExternalInput")
    qp_d = nc.dram_tensor("qp", [B, K], f32, kind="ExternalOutput")

    with tile.TileContext(nc) as tc:
        with (
            tc.tile_pool(name="ins", bufs=1) as in_pool,
            tc.tile_pool(name="psum", bufs=1, space="PSUM") as psum_pool,
            tc.tile_pool(name="work", bufs=1) as work_pool,
        ):
            xt0_t = in_pool.tile([CP, HS * B], fxt, name="xt0t")
            xt1_t = in_pool.tile([CP, HS * B], fxt, name="xt1t")
            ct0_t = in_pool.tile([CP, HS * K], fct, name="ct0t")
            ct1_t = in_pool.tile([CP, HS * K], fct, name="ct1t")
            # Balanced 2-queue schedule, bank-0 operands first on each.
            nc.sync.dma_start(out=xt0_t[:], in_=xt0_d.ap())
            nc.scalar.dma_start(out=ct0_t[:], in_=ct0_d.ap())
            nc.sync.dma_start(out=ct1_t[:], in_=ct1_d.ap())
            nc.scalar.dma_start(out=xt1_t[:], in_=xt1_d.ap())

            pss = [
                psum_pool.tile([128, TB * K], f32, name=f"ps{b}")
                for b in range(NBANK)
            ]
            dis = [
                work_pool.tile([128, TB, K], fdi, name=f"di{b}")
                for b in range(NBANK)
            ]

            xts = (xt0_t, xt1_t)
            cts = (ct0_t, ct1_t)
            for s in range(SLOC):
                b, u = divmod(s, TB)
                h, sh = divmod(s, HS)
                nc.tensor.matmul(
                    pss[b][:, u * K:(u + 1) * K],
                    lhsT=xts[h][:, sh * B:(sh + 1) * B],
                    rhs=cts[h][:, sh * K:(sh + 1) * K],
                    start=True,
                    stop=True,
                )

            # per-bank: sqrt (ACT), contiguous fp16 add-tree + running
            # accumulator (DVE)
            acc = None
            for b in range(NBANK):
                nc.scalar.activation(
                    dis[b][:], pss[b][:], mybir.ActivationFunctionType.Sqrt
                )
                tb = work_pool.tile([128, 2, K], fdi, name=f"tb{b}")
                nc.vector.tensor_tensor(
                    tb[:], dis[b][:, 0:2, :], dis[b][:, 2:4, :],
                    op=mybir.AluOpType.add,
                )
                pb = work_pool.tile([128, K], fdi, name=f"pb{b}")
                nc.vector.tensor_tensor(
                    pb[:], tb[:, 0, :], tb[:, 1, :], op=mybir.AluOpType.add
                )
                if acc is None:
                    acc = pb
                elif b < NBANK - 1:
                    nacc = work_pool.tile([128, K], fdi, name=f"acc{b}")
                    nc.vector.tensor_tensor(
                        nacc[:], acc[:], pb[:], op=mybir.AluOpType.add
                    )
                    acc = nacc
                else:
                    qt = work_pool.tile([B, K], f32, name="qt")
                    nc.vector.tensor_tensor(
                        qt[:], acc[:], pb[:], op=mybir.AluOpType.add
                    )
                    acc = qt
            nc.sync.dma_start(out=qp_d.ap(), in_=acc[:])

    nc.compile()
    return nc


def _prep_inputs(x, centroids):
    """Host-side shard + transpose + augmentation. Returns in_maps list."""
    from concourse import mybir

    xt_np = mybir.dt.np(getattr(mybir.dt, XT_DT))
    ct_np = mybir.dt.np(getattr(mybir.dt, CT_DT))
    x = np.ascontiguousarray(np.asarray(x, dtype=np.float32)).reshape(B, S, F)
    c = np.ascontiguousarray(np.asarray(centroids, dtype=np.float32)).reshape(K, S, F)

    in_maps = []
    for i in range(NCORES):
        m = {}
        for h in range(2):
            sl = slice(i * SLOC + h * HS, i * SLOC + (h + 1) * HS)
            xs = x[:, sl, :]                          # [B, HS, F]
            xt = np.empty((CP, HS * B), dtype=xt_np)
            xt[:F] = xs.transpose(2, 1, 0).reshape(F, HS * B)
            xt[F] = 1.0
            xt[F + 1] = ((xs * xs).sum(-1, dtype=np.float32).T).reshape(HS * B)
            m[f"xt{h}"] = xt
            cs = c[:, sl, :]                          # [K, HS, F]
            ct = np.empty((CP, HS * K), dtype=ct_np)
            ct[:F] = (-2.0 * cs).transpose(2, 1, 0).reshape(F, HS * K)
            ct[F] = ((cs * cs).sum(-1, dtype=np.float32).T).reshape(HS * K)
            ct[F + 1] = 1.0
            m[f"ct{h}"] = ct
        in_maps.append(m)
    return in_maps


def kernel(x, centroids):
    from concourse.bass_utils import run_bass_kernel_spmd

    if "nc" not in _CACHE:
        _CACHE["nc"] = _build_nc()
    nc = _CACHE["nc"]

    in_maps = _prep_inputs(x, centroids)
    res = run_bass_kernel_spmd(nc, in_maps, core_ids=list(range(NCORES)))
    dist = np.zeros((B, K), dtype=np.float64)
    for i in range(NCORES):
        dist += res.results[i]["qp"].astype(np.float64)
    # q tail (exact, host): q = (1 + d^2/2)^-3 normalized over k
    q = 1.0 / (1.0 + dist * dist / 2.0)
    q = q * q * q
    q = q / q.sum(axis=1, keepdims=True)
    return q.astype(np.float32)


# revision 8
# speedup vs baseline: 1.1971x; 1.0041x over previous
"""Trainium2 Bass kernel for nn_Clustering_80900003987951 (vq_codebook).

Math (reference):
  x: [B=128, S=128, F=64, 1], centroids: [1, K=64, S=128, F=64]
  d2[b,k,s] = sum_f (x[b,s,f] - c[k,s,f])^2
  dist[b,k] = sum_s sqrt(d2[b,k,s])
  q = (1 + dist^2/2)^-3, normalized over k                  -> [B, K]

Strategy: shard the SEQUENCE dim across the 8 cores (S_loc=16), keep the
full batch on every core. Per-core input drops to ~200KB (vs 1.36MB for
batch sharding, where every core must load all centroids), matmuls use
all 128 output partitions, and the device returns the per-core partial
  qp[b,k] = sum_{s in shard} sqrt(d2[b,k,s])     [128, 64] f32
The host sums the 8 partials and applies the tiny q tail (25K flops,
~0.002% of the work) exactly in float64.

Device pipeline per core:
  xt [66, S_loc*B]: rows 0-63 = x^T (F on partitions), 64 = 1, 65 = |x|^2
  ct [66, S_loc*K]: rows 0-63 = -2*c^T, 64 = |c|^2, 65 = 1
  per s: d2 tile = xt_s^T @ ct_s -> PSUM [128,64], ONE fp8 matmul per s
  (uniform weight dtype — alternating fp8/fp16 weights measured 325ns/s
  because it breaks ldweights/matmul pipelining, vs 60ns uniform; the
  part of the fp8 quantization error that is constant across k cancels
  in the normalized output, measured total ~a few e-3 vs 2e-2 budget).
  16 s split over 4 PSUM banks, skewed (5,5,4,2) so only sqrt[128,128]
  + 2 small DVE adds remain after the last matmul; ACT sqrt per bank ->
  fp16 sbuf; contiguous fp16 add-trees + running accumulator on DVE for
  the s-sum (a strided tensor_reduce measured 1.8ns/elem vs ~0.5 here).
DMA notes: per-queue throughput is ~85GB/s and only SP/Activation issue
usable HWDGE queues (gpsimd SWDGE moved 64KB in ~4us). Transfers split
at s=8 and interleaved so early-bank operands land first:
  sync:   xt0 (s0-7, 67KB)  then ct1 (s8-15, 34KB)
  scalar: ct0 (s0-7, 34KB)  then xt1 (s8-15, 67KB)
"""

import numpy as np

B, K, S, F = 128, 64, 128, 64
NCORES = 8
SLOC = S // NCORES          # 16 sequence positions per core
BANKS = (5, 5, 4, 2)        # skewed psum banks; short final bank
CP = F + 2                  # 66 contraction rows (data + aug)
HS = SLOC // 2              # 8: s-positions per transfer piece

XT_DT = "float8e4"
CT_DT = "float8e4"
DI_DT = "float16"           # sqrt results + partial sums; 2x DVE throughput

_CACHE = {}


def _build_nc():
    import concourse.bacc as bacc
    import concourse.tile as tile
    from concourse import mybir

    f32 = mybir.dt.float32
    fxt = getattr(mybir.dt, XT_DT)
    fct = getattr(mybir.dt, CT_DT)
    fdi = getattr(mybir.dt, DI_DT)
    nc = bacc.Bacc("TRN2", target_bir_lowering=False, debug=False)

    xt0_d = nc.dram_tensor("xt0", [CP, HS * B], fxt, kind="ExternalInput")
    xt1_d = nc.dram_tensor("xt1", [CP, HS * B], fxt, kind="ExternalInput")
    ct0_d = nc.dram_tensor("ct0", [CP, HS * K], fct, kind="ExternalInput")
    ct1_d = nc.dram_tensor("ct1", [CP, HS * K], fct, kind="ExternalInput")
    qp_d = nc.dram_tensor("qp", [B, K], f32, kind="ExternalOutput")

    with tile.TileContext(nc) as tc:
        with (
            tc.tile_pool(name="ins", bufs=1) as in_pool,
            tc.tile_pool(name="psum", bufs=1, space="PSUM") as psum_pool,
            tc.tile_pool(name="work", bufs=1) as work_pool,
        ):
            xt0_t = in_pool.tile([CP, HS * B], fxt, name="xt0t")
            xt1_t = in_pool.tile([CP, HS * B], fxt, name="xt1t")
            ct0_t = in_pool.tile([CP, HS * K], fct, name="ct0t")
            ct1_t = in_pool.tile([CP, HS * K], fct, name="ct1t")
            # Balanced 2-queue schedule, early-bank operands first on each.
            nc.sync.dma_start(out=xt0_t[:], in_=xt0_d.ap())
            nc.scalar.dma_start(out=ct0_t[:], in_=ct0_d.ap())
            nc.sync.dma_start(out=ct1_t[:], in_=ct1_d.ap())
            nc.scalar.dma_start(out=xt1_t[:], in_=xt1_d.ap())

            pss = [
                psum_pool.tile([128, t * K], f32, name=f"ps{b}")
                for b, t in enumerate(BANKS)
            ]
            dis = [
                work_pool.tile([128, t, K], fdi, name=f"di{b}")
                for b, t in enumerate(BANKS)
            ]

            xts = (xt0_t, xt1_t)
            cts = (ct0_t, ct1_t)
            s = 0
            for b, t in enumerate(BANKS):
                for u in range(t):
                    h, sh = divmod(s, HS)
                    nc.tensor.matmul(
                        pss[b][:, u * K:(u + 1) * K],
                        lhsT=xts[h][:, sh * B:(sh + 1) * B],
                        rhs=cts[h][:, sh * K:(sh + 1) * K],
                        start=True,
                        stop=True,
                    )
                    s += 1

            # per-bank: sqrt (ACT) + contiguous fp16 add-tree, then a
            # running accumulator (DVE)
            def bank_tree(b, t):
                nc.scalar.activation(
                    dis[b][:], pss[b][:], mybir.ActivationFunctionType.Sqrt
                )
                d = dis[b]
                if t == 2:
                    pb = work_pool.tile([128, K], fdi, name=f"pb{b}")
                    nc.vector.tensor_tensor(
                        pb[:], d[:, 0, :], d[:, 1, :], op=mybir.AluOpType.add
                    )
                    return pb
                # t in (4, 5): pairwise halves then fold the odd tail
                tb = work_pool.tile([128, 2, K], fdi, name=f"tb{b}")
                nc.vector.tensor_tensor(
                    tb[:], d[:, 0:2, :], d[:, 2:4, :], op=mybir.AluOpType.add
                )
                pb = work_pool.tile([128, K], fdi, name=f"pb{b}")
                nc.vector.tensor_tensor(
                    pb[:], tb[:, 0, :], tb[:, 1, :], op=mybir.AluOpType.add
                )
                if t == 5:
                    pb5 = work_pool.tile([128, K], fdi, name=f"pb5{b}")
                    nc.vector.tensor_tensor(
                        pb5[:], pb[:], d[:, 4, :], op=mybir.AluOpType.add
                    )
                    pb = pb5
                return pb

            acc = None
            for b, t in enumerate(BANKS):
                pb = bank_tree(b, t)
                if acc is None:
                    acc = pb
                elif b < len(BANKS) - 1:
                    nacc = work_pool.tile([128, K], fdi, name=f"acc{b}")
                    nc.vector.tensor_tensor(
                        nacc[:], acc[:], pb[:], op=mybir.AluOpType.add
                    )
                    acc = nacc
                else:
                    qt = work_pool.tile([B, K], f32, name="qt")
                    nc.vector.tensor_tensor(
                        qt[:], acc[:], pb[:], op=mybir.AluOpType.add
                    )
                    acc = qt
            nc.sync.dma_start(out=qp_d.ap(), in_=acc[:])

    nc.compile()
    return nc


def _prep_inputs(x, centroids):
    """Host-side shard + transpose + augmentation. Returns in_maps list."""
    from concourse import mybir

    xt_np = mybir.dt.np(getattr(mybir.dt, XT_DT))
    ct_np = mybir.dt.np(getattr(mybir.dt, CT_DT))
    x = np.ascontiguousarray(np.asarray(x, dtype=np.float32)).reshape(B, S, F)
    c = np.ascontiguousarray(np.asarray(centroids, dtype=np.float32)).reshape(K, S, F)

    in_maps = []
    for i in range(NCORES):
        m = {}
        for h in range(2):
            sl = slice(i * SLOC + h * HS, i * SLOC + (h + 1) * HS)
            xs = x[:, sl, :]                          # [B, HS, F]
            xt = np.empty((CP, HS * B), dtype=np.float32)
            xt[:F] = xs.transpose(2, 1, 0).reshape(F, HS * B)
            xt[F] = 1.0
            xt[F + 1] = ((xs * xs).sum(-1, dtype=np.float32).T).reshape(HS * B)
            m[f"xt{h}"] = xt.astype(xt_np)
            cs = c[:, sl, :]                          # [K, HS, F]
            ct = np.empty((CP, HS * K), dtype=np.float32)
            ct[:F] = (-2.0 * cs).transpose(2, 1, 0).reshape(F, HS * K)
            ct[F] = ((cs * cs).sum(-1, dtype=np.float32).T).reshape(HS * K)
            ct[F + 1] = 1.0
            m[f"ct{h}"] = ct.astype(ct_np)
        in_maps.append(m)
    return in_maps


def kernel(x, centroids):
    from concourse.bass_utils import run_bass_kernel_spmd

    if "nc" not in _CACHE:
        _CACHE["nc"] = _build_nc()
    nc = _CACHE["nc"]

    in_maps = _prep_inputs(x, centroids)
    res = run_bass_kernel_spmd(nc, in_maps, core_ids=list(range(NCORES)))
    dist = np.zeros((B, K), dtype=np.float64)
    for i in range(NCORES):
        dist += res.results[i]["qp"].astype(np.float64)
    # q tail (exact, host): q = (1 + d^2/2)^-3 normalized over k
    q = 1.0 / (1.0 + dist * dist / 2.0)
    q = q * q * q
    q = q / q.sum(axis=1, keepdims=True)
    return q.astype(np.float32)


# revision 9
# speedup vs baseline: 1.2102x; 1.0109x over previous
"""Trainium2 Bass kernel for nn_Clustering_80900003987951 (vq_codebook).

Math (reference):
  x: [B=128, S=128, F=64, 1], centroids: [1, K=64, S=128, F=64]
  d2[b,k,s] = sum_f (x[b,s,f] - c[k,s,f])^2
  dist[b,k] = sum_s sqrt(d2[b,k,s])
  q = (1 + dist^2/2)^-3, normalized over k                  -> [B, K]

Strategy: shard the SEQUENCE dim across the 8 cores (S_loc=16), keep the
full batch on every core. Per-core input drops to ~200KB (vs 1.36MB for
batch sharding, where every core must load all centroids), matmuls use
all 128 output partitions, and the device returns two per-core partial
sums of sqrt(d2) over its s-shard (split so the final DMA depends only
on the short last PSUM bank). The host sums the 16 partials and applies
the tiny q tail (25K flops, ~0.002% of the work) exactly in float64.

Device pipeline per core:
  xt [66, S_loc*B]: rows 0-63 = x^T (F on partitions), 64 = 1, 65 = |x|^2
  ct [66, S_loc*K]: rows 0-63 = -2*c^T, 64 = |c|^2, 65 = 1
  per s: d2 tile = xt_s^T @ ct_s -> PSUM [128,64], ONE fp8 matmul per s
  (uniform weight dtype — alternating fp8/fp16 weights measured 325ns/s
  because it breaks ldweights/matmul pipelining, vs 60ns uniform; the
  part of the fp8 quantization error that is constant across k cancels
  in the normalized output, total measured ~a few e-3 vs 2e-2 budget).
  16 s split over 4 PSUM banks, skewed (5,5,4,2) so after the final
  matmul only sqrt[128,128] + one DVE pair-add + its own DMA remain; ACT
  sqrt per bank -> fp16 sbuf; contiguous fp16 add-trees + accumulator on
  DVE (strided tensor_reduce measured 1.8ns/elem vs ~0.5 here). A dummy
  activation is emitted before the scalar-engine DMA issues so both ACT
  table loads run back-to-back at body start instead of gating sqrt0.
DMA notes: per-queue throughput is ~85GB/s and only SP/Activation issue
usable HWDGE queues (gpsimd SWDGE moved 64KB in ~4us). Transfers split
at s=5 (bank-0 boundary) and interleaved so bank 0 starts ~1us earlier:
  sync:   xt0 (s0-4, 42KB)  then ct1 (s5-15, 46KB), then qp1 out
  scalar: ct0 (s0-4, 21KB)  then xt1 (s5-15, 92KB), then qp0 out
"""

import numpy as np

B, K, S, F = 128, 64, 128, 64
NCORES = 8
SLOC = S // NCORES          # 16 sequence positions per core
BANKS = (5, 5, 4, 2)        # skewed psum banks; short final bank
CP = F + 2                  # 66 contraction rows (data + aug)
HS = BANKS[0]               # 5: s-positions in the first transfer piece

XT_DT = "float8e4"
CT_DT = "float8e4"
DI_DT = "float16"           # sqrt results + partial sums; 2x DVE throughput

_CACHE = {}


def _build_nc():
    import concourse.bacc as bacc
    import concourse.tile as tile
    from concourse import mybir

    f32 = mybir.dt.float32
    fxt = getattr(mybir.dt, XT_DT)
    fct = getattr(mybir.dt, CT_DT)
    fdi = getattr(mybir.dt, DI_DT)
    nc = bacc.Bacc("TRN2", target_bir_lowering=False, debug=False)

    S1 = SLOC - HS          # 11 s-positions in the second piece
    xt0_d = nc.dram_tensor("xt0", [CP, HS * B], fxt, kind="ExternalInput")
    xt1_d = nc.dram_tensor("xt1", [CP, S1 * B], fxt, kind="ExternalInput")
    ct0_d = nc.dram_tensor("ct0", [CP, HS * K], fct, kind="ExternalInput")
    ct1_d = nc.dram_tensor("ct1", [CP, S1 * K], fct, kind="ExternalInput")
    qp0_d = nc.dram_tensor("qp0", [B, K], fdi, kind="ExternalOutput")
    qp1_d = nc.dram_tensor("qp1", [B, K], fdi, kind="ExternalOutput")

    with tile.TileContext(nc) as tc:
        with (
            tc.tile_pool(name="ins", bufs=1) as in_pool,
            tc.tile_pool(name="psum", bufs=1, space="PSUM") as psum_pool,
            tc.tile_pool(name="work", bufs=1) as work_pool,
        ):
            # Dummy activation first: pulls both ACT table loads to the
            # top of the scalar stream, ahead of its DMA issues.
            dm = work_pool.tile([1, 2], f32, name="dm")
            nc.vector.memset(dm[:], 1.0)
            dm2 = work_pool.tile([1, 2], f32, name="dm2")
            nc.scalar.activation(
                dm2[:], dm[:], mybir.ActivationFunctionType.Sqrt
            )

            xt0_t = in_pool.tile([CP, HS * B], fxt, name="xt0t")
            xt1_t = in_pool.tile([CP, S1 * B], fxt, name="xt1t")
            ct0_t = in_pool.tile([CP, HS * K], fct, name="ct0t")
            ct1_t = in_pool.tile([CP, S1 * K], fct, name="ct1t")
            # Balanced 2-queue schedule, bank-0 operands first on each.
            nc.sync.dma_start(out=xt0_t[:], in_=xt0_d.ap())
            nc.scalar.dma_start(out=ct0_t[:], in_=ct0_d.ap())
            nc.sync.dma_start(out=ct1_t[:], in_=ct1_d.ap())
            nc.scalar.dma_start(out=xt1_t[:], in_=xt1_d.ap())

            pss = [
                psum_pool.tile([128, t * K], f32, name=f"ps{b}")
                for b, t in enumerate(BANKS)
            ]
            dis = [
                work_pool.tile([128, t, K], fdi, name=f"di{b}")
                for b, t in enumerate(BANKS)
            ]

            s = 0
            for b, t in enumerate(BANKS):
                for u in range(t):
                    if s < HS:
                        xt_t, ct_t, sh = xt0_t, ct0_t, s
                    else:
                        xt_t, ct_t, sh = xt1_t, ct1_t, s - HS
                    nc.tensor.matmul(
                        pss[b][:, u * K:(u + 1) * K],
                        lhsT=xt_t[:, sh * B:(sh + 1) * B],
                        rhs=ct_t[:, sh * K:(sh + 1) * K],
                        start=True,
                        stop=True,
                    )
                    s += 1

            # per-bank: sqrt (ACT) + contiguous fp16 add-tree (DVE)
            def bank_tree(b, t):
                nc.scalar.activation(
                    dis[b][:], pss[b][:], mybir.ActivationFunctionType.Sqrt
                )
                d = dis[b]
                if t == 2:
                    pb = work_pool.tile([128, K], fdi, name=f"pb{b}")
                    nc.vector.tensor_tensor(
                        pb[:], d[:, 0, :], d[:, 1, :], op=mybir.AluOpType.add
                    )
                    return pb
                # t in (4, 5): pairwise halves then fold the odd tail
                tb = work_pool.tile([128, 2, K], fdi, name=f"tb{b}")
                nc.vector.tensor_tensor(
                    tb[:], d[:, 0:2, :], d[:, 2:4, :], op=mybir.AluOpType.add
                )
                pb = work_pool.tile([128, K], fdi, name=f"pb{b}")
                nc.vector.tensor_tensor(
                    pb[:], tb[:, 0, :], tb[:, 1, :], op=mybir.AluOpType.add
                )
                if t == 5:
                    pb5 = work_pool.tile([128, K], fdi, name=f"pb5{b}")
                    nc.vector.tensor_tensor(
                        pb5[:], pb[:], d[:, 4, :], op=mybir.AluOpType.add
                    )
                    pb = pb5
                return pb

            # banks 0-2 fold into qp0 (scalar queue, overlapped); the
            # terminal chain is only: sqrt(bank3) -> pair add -> qp1 DMA.
            pb0 = bank_tree(0, BANKS[0])
            pb1 = bank_tree(1, BANKS[1])
            a01 = work_pool.tile([128, K], fdi, name="a01")
            nc.vector.tensor_tensor(
                a01[:], pb0[:], pb1[:], op=mybir.AluOpType.add
            )
            pb2 = bank_tree(2, BANKS[2])
            a012 = work_pool.tile([128, K], fdi, name="a012")
            nc.vector.tensor_tensor(
                a012[:], a01[:], pb2[:], op=mybir.AluOpType.add
            )
            nc.scalar.dma_start(out=qp0_d.ap(), in_=a012[:])
            pb3 = bank_tree(3, BANKS[3])
            nc.sync.dma_start(out=qp1_d.ap(), in_=pb3[:])

    nc.compile()
    return nc


def _prep_inputs(x, centroids):
    """Host-side shard + transpose + augmentation. Returns in_maps list."""
    from concourse import mybir

    xt_np = mybir.dt.np(getattr(mybir.dt, XT_DT))
    ct_np = mybir.dt.np(getattr(mybir.dt, CT_DT))
    x = np.ascontiguousarray(np.asarray(x, dtype=np.float32)).reshape(B, S, F)
    c = np.ascontiguousarray(np.asarray(centroids, dtype=np.float32)).reshape(K, S, F)

    in_maps = []
    for i in range(NCORES):
        m = {}
        for h, (lo, hi) in enumerate(((0, HS), (HS, SLOC))):
            n = hi - lo
            sl = slice(i * SLOC + lo, i * SLOC + hi)
            xs = x[:, sl, :]                          # [B, n, F]
            xt = np.empty((CP, n * B), dtype=np.float32)
            xt[:F] = xs.transpose(2, 1, 0).reshape(F, n * B)
            xt[F] = 1.0
            xt[F + 1] = ((xs * xs).sum(-1, dtype=np.float32).T).reshape(n * B)
            m[f"xt{h}"] = xt.astype(xt_np)
            cs = c[:, sl, :]                          # [K, n, F]
            ct = np.empty((CP, n * K), dtype=np.float32)
            ct[:F] = (-2.0 * cs).transpose(2, 1, 0).reshape(F, n * K)
            ct[F] = ((cs * cs).sum(-1, dtype=np.float32).T).reshape(n * K)
            ct[F + 1] = 1.0
            m[f"ct{h}"] = ct.astype(ct_np)
        in_maps.append(m)
    return in_maps


def kernel(x, centroids):
    from concourse.bass_utils import run_bass_kernel_spmd

    if "nc" not in _CACHE:
        _CACHE["nc"] = _build_nc()
    nc = _CACHE["nc"]

    in_maps = _prep_inputs(x, centroids)
    res = run_bass_kernel_spmd(nc, in_maps, core_ids=list(range(NCORES)))
    dist = np.zeros((B, K), dtype=np.float64)
    for i in range(NCORES):
        dist += res.results[i]["qp0"].astype(np.float64)
        dist += res.results[i]["qp1"].astype(np.float64)
    # q tail (exact, host): q = (1 + d^2/2)^-3 normalized over k
    q = 1.0 / (1.0 + dist * dist / 2.0)
    q = q * q * q
    q = q / q.sum(axis=1, keepdims=True)
    return q.astype(np.float32)


# revision 10
# speedup vs baseline: 1.2475x; 1.0308x over previous
"""Trainium2 Bass kernel for nn_Clustering_80900003987951 (vq_codebook).

Math (reference):
  x: [B=128, S=128, F=64, 1], centroids: [1, K=64, S=128, F=64]
  d2[b,k,s] = sum_f (x[b,s,f] - c[k,s,f])^2
  dist[b,k] = sum_s sqrt(d2[b,k,s])
  q = (1 + dist^2/2)^-3, normalized over k                  -> [B, K]

Strategy: shard the SEQUENCE dim across the 8 cores (S_loc=16), keep the
full batch on every core. Per-core input drops to ~200KB (vs 1.36MB for
batch sharding, where every core must load all centroids), matmuls use
all 128 output partitions, and the device returns two per-core partial
sums of sqrt(d2) over its s-shard (split so the final DMA depends only
on the short last PSUM bank). The host sums the 16 partials and applies
the tiny q tail (25K flops, ~0.002% of the work) exactly in float64.

Device pipeline per core:
  xt [66, *]: rows 0-63 = x^T (F on partitions), 64 = 1, 65 = |x|^2
  ct [66, *]: rows 0-63 = -2*c^T, 64 = |c|^2, 65 = 1
  per s: d2 tile = xt_s^T @ ct_s -> PSUM [128,64], ONE fp8 matmul per s
  (uniform weight dtype — alternating fp8/fp16 weights measured 325ns/s
  because it breaks ldweights/matmul pipelining, vs 60ns uniform; the
  part of the fp8 quantization error that is constant across k cancels
  in the normalized output; measured 6e-3 vs the 2e-2 budget).
  16 s split over 4 PSUM banks, skewed (5,5,4,2) so after the final
  matmul only sqrt[128,128] + one DVE pair-add + its own DMA remain; ACT
  sqrt per bank -> fp16 sbuf; contiguous fp16 add-trees + accumulator on
  DVE (strided tensor_reduce measured 1.8ns/elem vs ~0.5 here). A dummy
  activation pulls the ACT table loads ahead of sqrt0.
DMA notes: per-queue throughput is ~85GB/s, issue costs ~0.8-1.6us of
sequencer time per dma_start (so 2 transfers/queue max), and only
SP/Activation issue usable HWDGE queues (gpsimd SWDGE moved 64KB in
~4us). Byte-balanced schedule, bank-0 operands first on each queue:
  sync:   A = xt(s0-4)  42KB   then  B = xt(s5-12)           67KB
  scalar: C = ct(s0-4)  21KB   then  D = ct(s5-15)|xt(s13-15) 71KB
"""

import numpy as np

B, K, S, F = 128, 64, 128, 64
NCORES = 8
SLOC = S // NCORES          # 16 sequence positions per core
BANKS = (5, 5, 4, 2)        # skewed psum banks; short final bank
CP = F + 2                  # 66 contraction rows (data + aug)
P0 = 5                      # s-positions in the first piece (bank 0)
XB = 13                     # xt(s5-12) in B; xt(s13-15) rides in D

X8_DT = "float8e4"
DI_DT = "float16"           # sqrt results + partial sums; 2x DVE throughput

_CACHE = {}

# D column layout (fp8): ct blocks for s5-15, then xt blocks for s13-15
D_CT_N = SLOC - P0          # 11 ct blocks of K
D_XT_OFF = D_CT_N * K       # 704
D_COLS = D_XT_OFF + (SLOC - XB) * B  # 704 + 384 = 1088


def _operand_layout(s):
    """Returns ((xt_tensor, xt_col), (ct_tensor, ct_col)) for position s."""
    if s < P0:
        xt = ("A", s * B)
        ct = ("C", s * K)
    else:
        ct = ("D", (s - P0) * K)
        if s < XB:
            xt = ("B", (s - P0) * B)
        else:
            xt = ("D", D_XT_OFF + (s - XB) * B)
    return xt, ct


def _build_nc():
    import concourse.bacc as bacc
    import concourse.tile as tile
    from concourse import mybir

    f32 = mybir.dt.float32
    f8 = getattr(mybir.dt, X8_DT)
    fdi = getattr(mybir.dt, DI_DT)
    nc = bacc.Bacc("TRN2", target_bir_lowering=False, debug=False)

    a_d = nc.dram_tensor("A", [CP, P0 * B], f8, kind="ExternalInput")
    b_d = nc.dram_tensor("B", [CP, (XB - P0) * B], f8, kind="ExternalInput")
    c_d = nc.dram_tensor("C", [CP, P0 * K], f8, kind="ExternalInput")
    d_d = nc.dram_tensor("D", [CP, D_COLS], f8, kind="ExternalInput")
    qp0_d = nc.dram_tensor("qp0", [B, K], fdi, kind="ExternalOutput")
    qp1_d = nc.dram_tensor("qp1", [B, K], fdi, kind="ExternalOutput")

    with tile.TileContext(nc) as tc:
        with (
            tc.tile_pool(name="ins", bufs=1) as in_pool,
            tc.tile_pool(name="psum", bufs=1, space="PSUM") as psum_pool,
            tc.tile_pool(name="work", bufs=1) as work_pool,
        ):
            # Dummy activation first: pulls the ACT table loads to the
            # top of the scalar stream, ahead of sqrt0's need.
            dm = work_pool.tile([1, 2], f32, name="dm")
            nc.vector.memset(dm[:], 1.0)
            dm2 = work_pool.tile([1, 2], f32, name="dm2")
            nc.scalar.activation(
                dm2[:], dm[:], mybir.ActivationFunctionType.Sqrt
            )

            tiles = {
                "A": in_pool.tile([CP, P0 * B], f8, name="At"),
                "B": in_pool.tile([CP, (XB - P0) * B], f8, name="Bt"),
                "C": in_pool.tile([CP, P0 * K], f8, name="Ct"),
                "D": in_pool.tile([CP, D_COLS], f8, name="Dt"),
            }
            nc.sync.dma_start(out=tiles["A"][:], in_=a_d.ap())
            nc.scalar.dma_start(out=tiles["C"][:], in_=c_d.ap())
            nc.sync.dma_start(out=tiles["B"][:], in_=b_d.ap())
            nc.scalar.dma_start(out=tiles["D"][:], in_=d_d.ap())

            pss = [
                psum_pool.tile([128, t * K], f32, name=f"ps{b}")
                for b, t in enumerate(BANKS)
            ]
            dis = [
                work_pool.tile([128, t, K], fdi, name=f"di{b}")
                for b, t in enumerate(BANKS)
            ]

            s = 0
            for b, t in enumerate(BANKS):
                for u in range(t):
                    (xn, xo), (cn, co) = _operand_layout(s)
                    nc.tensor.matmul(
                        pss[b][:, u * K:(u + 1) * K],
                        lhsT=tiles[xn][:, xo:xo + B],
                        rhs=tiles[cn][:, co:co + K],
                        start=True,
                        stop=True,
                    )
                    s += 1

            # per-bank: sqrt (ACT) + contiguous fp16 add-tree (DVE)
            def bank_tree(b, t):
                nc.scalar.activation(
                    dis[b][:], pss[b][:], mybir.ActivationFunctionType.Sqrt
                )
                d = dis[b]
                if t == 2:
                    pb = work_pool.tile([128, K], fdi, name=f"pb{b}")
                    nc.vector.tensor_tensor(
                        pb[:], d[:, 0, :], d[:, 1, :], op=mybir.AluOpType.add
                    )
                    return pb
                # t in (4, 5): pairwise halves then fold the odd tail
                tb = work_pool.tile([128, 2, K], fdi, name=f"tb{b}")
                nc.vector.tensor_tensor(
                    tb[:], d[:, 0:2, :], d[:, 2:4, :], op=mybir.AluOpType.add
                )
                pb = work_pool.tile([128, K], fdi, name=f"pb{b}")
                nc.vector.tensor_tensor(
                    pb[:], tb[:, 0, :], tb[:, 1, :], op=mybir.AluOpType.add
                )
                if t == 5:
                    pb5 = work_pool.tile([128, K], fdi, name=f"pb5{b}")
                    nc.vector.tensor_tensor(
                        pb5[:], pb[:], d[:, 4, :], op=mybir.AluOpType.add
                    )
                    pb = pb5
                return pb

            # banks 0-2 fold into qp0 (scalar queue, overlapped); the
            # terminal chain is only: sqrt(bank3) -> pair add -> qp1 DMA.
            pb0 = bank_tree(0, BANKS[0])
            pb1 = bank_tree(1, BANKS[1])
            a01 = work_pool.tile([128, K], fdi, name="a01")
            nc.vector.tensor_tensor(
                a01[:], pb0[:], pb1[:], op=mybir.AluOpType.add
            )
            pb2 = bank_tree(2, BANKS[2])
            a012 = work_pool.tile([128, K], fdi, name="a012")
            nc.vector.tensor_tensor(
                a012[:], a01[:], pb2[:], op=mybir.AluOpType.add
            )
            nc.scalar.dma_start(out=qp0_d.ap(), in_=a012[:])
            pb3 = bank_tree(3, BANKS[3])
            nc.sync.dma_start(out=qp1_d.ap(), in_=pb3[:])

    nc.compile()
    return nc


def _prep_inputs(x, centroids):
    """Host-side shard + transpose + augmentation. Returns in_maps list."""
    from concourse import mybir

    f8_np = mybir.dt.np(getattr(mybir.dt, X8_DT))
    x = np.ascontiguousarray(np.asarray(x, dtype=np.float32)).reshape(B, S, F)
    c = np.ascontiguousarray(np.asarray(centroids, dtype=np.float32)).reshape(K, S, F)

    in_maps = []
    for i in range(NCORES):
        # full per-core xt [66, SLOC*B] and ct [66, SLOC*K] in f32 first
        sl = slice(i * SLOC, (i + 1) * SLOC)
        xs = x[:, sl, :]                              # [B, SLOC, F]
        xt = np.empty((CP, SLOC * B), dtype=np.float32)
        xt[:F] = xs.transpose(2, 1, 0).reshape(F, SLOC * B)
        xt[F] = 1.0
        xt[F + 1] = ((xs * xs).sum(-1, dtype=np.float32).T).reshape(SLOC * B)
        cs = c[:, sl, :]                              # [K, SLOC, F]
        ct = np.empty((CP, SLOC * K), dtype=np.float32)
        ct[:F] = (-2.0 * cs).transpose(2, 1, 0).reshape(F, SLOC * K)
        ct[F] = ((cs * cs).sum(-1, dtype=np.float32).T).reshape(SLOC * K)
        ct[F + 1] = 1.0
        xt8 = xt.astype(f8_np)
        ct8 = ct.astype(f8_np)

        dmat = np.empty((CP, D_COLS), dtype=f8_np)
        dmat[:, :D_XT_OFF] = ct8[:, P0 * K:]
        dmat[:, D_XT_OFF:] = xt8[:, XB * B:]
        in_maps.append({
            "A": np.ascontiguousarray(xt8[:, :P0 * B]),
            "B": np.ascontiguousarray(xt8[:, P0 * B:XB * B]),
            "C": np.ascontiguousarray(ct8[:, :P0 * K]),
            "D": dmat,
        })
    return in_maps


def kernel(x, centroids):
    from concourse.bass_utils import run_bass_kernel_spmd

    if "nc" not in _CACHE:
        _CACHE["nc"] = _build_nc()
    nc = _CACHE["nc"]

    in_maps = _prep_inputs(x, centroids)
    res = run_bass_kernel_spmd(nc, in_maps, core_ids=list(range(NCORES)))
    dist = np.zeros((B, K), dtype=np.float64)
    for i in range(NCORES):
        dist += res.results[i]["qp0"].astype(np.float64)
        dist += res.results[i]["qp1"].astype(np.float64)
    # q tail (exact, host): q = (1 + d^2/2)^-3 normalized over k
    q = 1.0 / (1.0 + dist * dist / 2.0)
    q = q * q * q
    q = q / q.sum(axis=1, keepdims=True)
    return q.astype(np.float32)
